# revision 19
# baseline (speedup 1.0000x reference)
"""Trainium2 Bass kernel for CrossModalAttentionScorer.

Contract: kernel(**inputs) takes FULL inputs (B=64), returns (x, scores_raw)
matching reference.py. Data-parallel over batch across 8 NeuronCores, dense
weights replicated.

Per-core layout strategy (B_local = 8 batches, processed in 4 pairs):
  - Host pre-transposes anchor ([A,D]->[D,A]) and query ([T,Dq]->[Dq,T]) so
    every matmul contraction lands on the SBUF partition dim; no on-device
    input transposes.
  - l2-normalize folded into per-partition scales:
      s_q applied at Q_proj / scoresT PSUM evictions (t on partitions there),
      s_a applied at the scores transpose eviction (a on partitions there).
  - scoresT [t, a] computed first (N=256 rhs), then PE-transposed to [a, t]
    for softmax + the scores_raw output.
  - Weight-shared matmuls (Wr, Wq, W1) pair two batches in the rhs free dim
    (N=512) to amortize LDWEIGHTS and stay in fp32r 1-cycle/row regime.
"""

import os
import numpy as np

import concourse.bass as bass
import concourse.mybir as mybir
import concourse.tile as tile
from concourse.masks import make_identity

F32 = mybir.dt.float32
F32R = mybir.dt.float32r
I32 = mybir.dt.int32
AF = mybir.ActivationFunctionType
ALU = mybir.AluOpType
AX = mybir.AxisListType

N_CORES = 8
B, A, T, D, DQ, H = 64, 256, 128, 512, 768, 512
B_LOCAL = B // N_CORES


def _split_multi_waits(nc):
    """The walrus build here encodes at most one sync-wait per instruction.
    Tile emits instructions with several; hoist the extras into standalone
    event-semaphore waits on the same engine queue right before them."""
    for fn in nc.m.functions:
        for bb in fn.blocks:
            insts = list(bb.instructions)
            out = []
            changed = False
            for inst in insts:
                si = inst.sync_info
                if si is not None and si.on_wait and len(si.on_wait) > 1:
                    waits = list(si.on_wait)
                    for w in waits[:-1]:
                        ev = mybir.InstEventSemaphore(
                            name=f"splitwait_{nc.next_id()}",
                            engine=inst.engine,
                            ins=[], outs=[],
                            sync_info=mybir.SyncInfo(on_wait=[w], on_update=[]),
                        )
                        out.append(ev)
                    inst.sync_info = mybir.SyncInfo(
                        on_wait=[waits[-1]], on_update=list(si.on_update or []))
                    changed = True
                out.append(inst)
            if changed:
                bb.instructions = out


def build_module(b_local=B_LOCAL):
    nc = bass.Bass()

    anchor = nc.dram_tensor("anchor", [b_local, A, D], F32, kind="ExternalInput")
    anchor_t = nc.dram_tensor("anchor_t", [b_local, D, A], F32R, kind="ExternalInput")
    query = nc.dram_tensor("query", [b_local, T, DQ], F32, kind="ExternalInput")
    query_t = nc.dram_tensor("query_t", [b_local, DQ, T], F32R, kind="ExternalInput")
    mask = nc.dram_tensor("mask", [b_local, T], I32, kind="ExternalInput")
    Wr = nc.dram_tensor("Wr", [D, H], F32R, kind="ExternalInput")
    Wq = nc.dram_tensor("Wq", [DQ, H], F32R, kind="ExternalInput")
    W1 = nc.dram_tensor("W1", [D + 2 * H, H], F32R, kind="ExternalInput")
    b1 = nc.dram_tensor("b1", [H], F32, kind="ExternalInput")
    W2 = nc.dram_tensor("W2", [H, H], F32R, kind="ExternalInput")
    b2 = nc.dram_tensor("b2", [H], F32, kind="ExternalInput")
    x_out = nc.dram_tensor("x_out", [b_local, A, H], F32, kind="ExternalOutput")
    s_out = nc.dram_tensor("s_out", [b_local, A, T], F32, kind="ExternalOutput")

    KD, KQ, K1, KH = D // 128, DQ // 128, (D + 2 * H) // 128, H // 128  # 4,6,12,4

    with tile.TileContext(nc) as tc:
        from contextlib import ExitStack

        with ExitStack() as ctx:
            ec = ctx.enter_context
            consts = ec(tc.tile_pool(name="consts", bufs=1))
            pin = ec(tc.tile_pool(name="pin", bufs=2))        # pair input tiles
            pnat = ec(tc.tile_pool(name="pnat", bufs=3))      # natural-layout inputs
            pmid = ec(tc.tile_pool(name="pmid", bufs=1))      # pair intermediates
            psml = ec(tc.tile_pool(name="psml", bufs=6))      # small per-batch tiles
            pout = ec(tc.tile_pool(name="pout", bufs=4))      # output staging
            psA = ec(tc.tile_pool(name="psA", bufs=4, space="PSUM"))
            psB = ec(tc.tile_pool(name="psB", bufs=4, space="PSUM"))

            ident = consts.tile([128, 128], F32, tag="ident")
            make_identity(nc, ident[:])

            def load_w(t_hbm, kk, name):
                tiles = []
                for c in range(kk):
                    w = consts.tile([128, H], F32R, tag=f"{name}{c}")
                    nc.sync.dma_start(w[:], t_hbm[c * 128:(c + 1) * 128, :])
                    tiles.append(w)
                return tiles

            wr_sb = load_w(Wr, KD, "wr")
            wq_sb = load_w(Wq, KQ, "wq")
            w1_sb = load_w(W1, K1, "w1")
            w2_sb = load_w(W2, KH, "w2")

            b1_sb = consts.tile([128, KH], F32, tag="b1")
            nc.sync.dma_start(b1_sb[:], b1.rearrange("(m p) -> p m", p=128))
            b2_bc = consts.tile([128, H], F32, tag="b2")
            nc.sync.dma_start(b2_bc[:], b2[None, :].to_broadcast((128, H)))

            for pr in range(b_local // 2):
                # ---- pair input tiles ----
                atr = [pin.tile([128, 2 * A], F32R, tag=f"atr{c}", name=f"atr{c}")
                       for c in range(KD)]   # anchor_t pair tiles [128, 512]
                qtp = [pin.tile([128, 2 * T], F32R, tag=f"qtp{c}", name=f"qtp{c}")
                       for c in range(KQ)]   # query_t pair tiles [128, 256]

                s_a = [[None, None], [None, None]]
                s_q = [None, None]
                negb = [None, None]

                for j in range(2):
                    g = pr * 2 + j
                    for c in range(KD):
                        nc.sync.dma_start(
                            atr[c][:, j * A:(j + 1) * A],
                            anchor_t[g, c * 128:(c + 1) * 128, :])
                    for c in range(KQ):
                        nc.sync.dma_start(
                            qtp[c][:, j * T:(j + 1) * T],
                            query_t[g, c * 128:(c + 1) * 128, :])

                    # ---- norms (natural layout) ----
                    for i in range(2):
                        a_in = pnat.tile([128, D], F32, tag="a_in")
                        nc.sync.dma_start(a_in[:], anchor[g, i * 128:(i + 1) * 128, :])
                        ss = psml.tile([128, 1], F32, tag="ss_a")
                        nc.scalar.activation(a_in[:], a_in[:], AF.Square,
                                             accum_out=ss[:])
                        srt = psml.tile([128, 1], F32, tag="srt_a")
                        nc.scalar.activation(srt[:], ss[:], AF.Sqrt)
                        sa = psml.tile([128, 1], F32, tag="sa")
                        nc.vector.reciprocal(sa[:], srt[:])
                        s_a[j][i] = sa

                    q_in = pnat.tile([128, DQ], F32, tag="q_in")
                    nc.sync.dma_start(q_in[:], query[g, :, :])
                    ssq = psml.tile([128, 1], F32, tag="ss_q")
                    nc.scalar.activation(q_in[:], q_in[:], AF.Square,
                                         accum_out=ssq[:])
                    srtq = psml.tile([128, 1], F32, tag="srt_q")
                    nc.scalar.activation(srtq[:], ssq[:], AF.Sqrt)
                    sq = psml.tile([128, 1], F32, tag="sq")
                    nc.vector.reciprocal(sq[:], srtq[:])
                    s_q[j] = sq

                    # ---- mask -> additive bias row broadcast [128, T] ----
                    mbc = psml.tile([128, T], I32, tag="mbc")
                    nc.sync.dma_start(mbc[:], mask[g][None, :].to_broadcast((128, T)))
                    nb = psml.tile([128, T], F32, tag="negb")
                    nc.gpsimd.tensor_scalar(nb[:], mbc[:], 1.0e9, -1.0e9,
                                            op0=ALU.mult, op1=ALU.add)
                    negb[j] = nb

                # ---- U_RT = (anchor_raw @ Wr)^T, pair rhs N=512 ----
                urt = []
                for m in range(KH):
                    ps = psA.tile([128, 2 * A], F32, tag="psA")
                    for c in range(KD):
                        nc.tensor.matmul(ps[:], (wr_sb[c][:, m * 128:(m + 1) * 128]),
                                         (atr[c][:]), start=(c == 0), stop=(c == KD - 1))
                    u = pmid.tile([128, 2 * A], F32R, tag=f"urt{m}", name=f"urt{m}")
                    if m % 2 == 0:
                        nc.scalar.copy(u[:], ps[:])
                    else:
                        nc.vector.tensor_copy(u[:], ps[:])
                    urt.append(u)

                # ---- UQ_T = (query_raw @ Wq)^T, pair rhs N=256 ----
                uqt = []
                for m in range(KH):
                    ps = psB.tile([128, 2 * T], F32, tag="psB")
                    for c in range(KQ):
                        nc.tensor.matmul(ps[:], (wq_sb[c][:, m * 128:(m + 1) * 128]),
                                         (qtp[c][:]), start=(c == 0), stop=(c == KQ - 1))
                    u = pmid.tile([128, 2 * T], F32R, tag=f"uqt{m}", name=f"uqt{m}")
                    if m % 2 == 0:
                        nc.scalar.copy(u[:], ps[:])
                    else:
                        nc.vector.tensor_copy(u[:], ps[:])
                    uqt.append(u)

                # pair tiles filled per batch below
                attnT = pmid.tile([128, 2 * A], F32R, tag="attnT")
                attsb = [pmid.tile([128, 2 * A], F32R, tag=f"attsb{m}", name=f"attsb{m}")
                         for m in range(KH)]
                prod = [pmid.tile([128, 2 * A], F32R, tag=f"prod{m}", name=f"prod{m}")
                        for m in range(KD)]

                for j in range(2):
                    g = pr * 2 + j

                    # ---- Q_proj [t, h], normalized on eviction ----
                    ps_qp = psA.tile([128, H], F32, tag="psA")
                    for c in range(KQ):
                        nc.tensor.matmul(ps_qp[:], (qtp[c][:, j * T:(j + 1) * T]),
                                         (wq_sb[c][:]), start=(c == 0), stop=(c == KQ - 1))
                    qp = pnat.tile([128, H], F32R, tag="qp")
                    nc.scalar.activation(qp[:], ps_qp[:], AF.Identity, bias=0.0,
                                         scale=s_q[j][:])

                    # ---- scoresT [t, a], s_q folded on eviction ----
                    ps_st = psB.tile([128, A], F32, tag="psB")
                    for m in range(KH):
                        nc.tensor.matmul(ps_st[:], (uqt[m][:, j * T:(j + 1) * T]),
                                         (urt[m][:, j * A:(j + 1) * A]),
                                         start=(m == 0), stop=(m == KH - 1))
                    st_sb = psml.tile([128, A], F32, tag="st_sb")
                    nc.scalar.activation(st_sb[:], ps_st[:], AF.Identity, bias=0.0,
                                         scale=s_q[j][:])

                    # ---- transpose to [a, t]; s_a fold; mask; softmax ----
                    for i in range(2):
                        ps_t = psB.tile([128, 128], F32, tag="psB")
                        nc.tensor.transpose(ps_t[:], st_sb[:, i * 128:(i + 1) * 128], ident[:])
                        sc_t = psml.tile([128, T], F32, tag="sc_t")
                        nc.scalar.activation(sc_t[:], ps_t[:], AF.Copy, bias=0.0,
                                             scale=s_a[j][i][:])
                        sc = pout.tile([128, T], F32, tag="sc")
                        nc.vector.tensor_add(sc[:], sc_t[:], negb[j][:])
                        nc.sync.dma_start(s_out[g, i * 128:(i + 1) * 128, :], sc[:])

                        nmax = psml.tile([128, 1], F32, tag="nmax")
                        nc.vector.reduce_max(nmax[:], sc[:], axis=AX.X, negate=True)
                        pex = psml.tile([128, T], F32, tag="pex")
                        z = psml.tile([128, 1], F32, tag="z")
                        nc.scalar.activation(pex[:], sc[:], AF.Exp, bias=nmax[:],
                                             scale=1.0, accum_out=z[:])
                        rz = psml.tile([128, 1], F32, tag="rz")
                        nc.vector.reciprocal(rz[:], z[:])
                        at = psml.tile([128, T], F32, tag="at")
                        nc.vector.tensor_scalar_mul(at[:], pex[:], rz[:])

                        ps_t2 = psB.tile([128, 128], F32, tag="psB")
                        nc.tensor.transpose(ps_t2[:], at[:], ident[:])
                        nc.scalar.copy(attnT[:, j * A + i * 128: j * A + (i + 1) * 128],
                                       ps_t2[:])

                    # ---- attendedT [h, a] ----
                    for m in range(KH):
                        ps_at = psB.tile([128, A], F32, tag="psB")
                        nc.tensor.matmul(ps_at[:], (qp[:, m * 128:(m + 1) * 128]),
                                         (attnT[:, j * A:(j + 1) * A]),
                                         start=True, stop=True)
                        if m % 2 == 0:
                            nc.scalar.copy(attsb[m][:, j * A:(j + 1) * A], ps_at[:])
                        else:
                            nc.vector.tensor_copy(attsb[m][:, j * A:(j + 1) * A], ps_at[:])
                        nc.vector.tensor_mul(prod[m][:, j * A:(j + 1) * A],
                                             atr[m][:, j * A:(j + 1) * A], ps_at[:])

                # ---- xT = relu(W1^T @ combinedT + b1), pair rhs N=512 ----
                xt = []
                for m in range(KH):
                    ps_x = psA.tile([128, 2 * A], F32, tag="psA")
                    for c in range(K1):
                        rhs = atr[c] if c < KD else (attsb[c - KD] if c < KD + KH
                                                     else prod[c - KD - KH])
                        nc.tensor.matmul(ps_x[:], (w1_sb[c][:, m * 128:(m + 1) * 128]),
                                         (rhs[:]), start=(c == 0), stop=(c == K1 - 1))
                    xm = pmid.tile([128, 2 * A], F32R, tag=f"xt{m}", name=f"xt{m}")
                    if m % 2 == 0:
                        nc.scalar.activation(xm[:], ps_x[:], AF.Relu,
                                             bias=b1_sb[:, m:m + 1], scale=1.0)
                    else:
                        nc.vector.tensor_scalar(xm[:], ps_x[:], b1_sb[:, m:m + 1], 0.0,
                                                op0=ALU.add, op1=ALU.max)
                    xt.append(xm)

                # ---- x2 = xT^T @ W2 + b2 ----
                for j in range(2):
                    g = pr * 2 + j
                    for i in range(2):
                        ps_o = psA.tile([128, H], F32, tag="psA")
                        for m in range(KH):
                            nc.tensor.matmul(
                                ps_o[:],
                                (xt[m][:, j * A + i * 128: j * A + (i + 1) * 128]),
                                (w2_sb[m][:]), start=(m == 0), stop=(m == KH - 1))
                        ob = pout.tile([128, H], F32, tag="ob")
                        nc.vector.tensor_add(ob[:], ps_o[:], b2_bc[:])
                        nc.sync.dma_start(x_out[g, i * 128:(i + 1) * 128, :], ob[:])

    return nc


_NC_CACHE = {}


def _get_nc():
    if "nc" not in _NC_CACHE:
        nc = build_module()
        _split_multi_waits(nc)  # HW-only: CoreSim chokes on raw event-sems
        _NC_CACHE["nc"] = nc
    return _NC_CACHE["nc"]


LAST_RESULTS = None


def kernel(anchor_feats, query_embs, query_mask, Wr, Wq, W1, b1, W2, b2):
    global LAST_RESULTS
    from concourse.bass_utils import run_bass_kernel_spmd

    anchor_feats = np.ascontiguousarray(np.asarray(anchor_feats, dtype=np.float32))
    query_embs = np.ascontiguousarray(np.asarray(query_embs, dtype=np.float32))
    query_mask = np.ascontiguousarray(np.asarray(query_mask, dtype=np.int32))
    anchor_t = np.ascontiguousarray(anchor_feats.transpose(0, 2, 1))
    query_t = np.ascontiguousarray(query_embs.transpose(0, 2, 1))
    weights = {
        "Wr": np.ascontiguousarray(np.asarray(Wr, dtype=np.float32)),
        "Wq": np.ascontiguousarray(np.asarray(Wq, dtype=np.float32)),
        "W1": np.ascontiguousarray(np.asarray(W1, dtype=np.float32)),
        "b1": np.ascontiguousarray(np.asarray(b1, dtype=np.float32)),
        "W2": np.ascontiguousarray(np.asarray(W2, dtype=np.float32)),
        "b2": np.ascontiguousarray(np.asarray(b2, dtype=np.float32)),
    }

    in_maps = []
    for k in range(N_CORES):
        sl = slice(k * B_LOCAL, (k + 1) * B_LOCAL)
        in_maps.append({
            "anchor": anchor_feats[sl],
            "anchor_t": anchor_t[sl],
            "query": query_embs[sl],
            "query_t": query_t[sl],
            "mask": query_mask[sl],
            **weights,
        })

    nc = _get_nc()
    extra = {}
    if os.environ.get("BASS_TMPDIR"):
        extra["tmpdir"] = os.environ["BASS_TMPDIR"]
    res = run_bass_kernel_spmd(nc, in_maps, core_ids=list(range(N_CORES)), **extra)
    LAST_RESULTS = res
    x = np.concatenate([r["x_out"] for r in res.results], axis=0)
    s = np.concatenate([r["s_out"] for r in res.results], axis=0)
    return (x, s)


# revision 22
# speedup vs baseline: 1.0451x; 1.0451x over previous
"""Trainium2 Bass kernel for CrossModalAttentionScorer.

Contract: kernel(**inputs) takes FULL inputs (B=64), returns (x, scores_raw)
matching reference.py. Data-parallel over batch across 8 NeuronCores, dense
weights replicated.

Per-core layout strategy (B_local = 8 batches, processed in 4 pairs):
  - Host pre-transposes anchor ([A,D]->[D,A]) and query ([T,Dq]->[Dq,T]) so
    every matmul contraction lands on the SBUF partition dim; no on-device
    input transposes.
  - l2-normalize folded into per-partition scales:
      s_q applied at Q_proj / scoresT PSUM evictions (t on partitions there),
      s_a applied at the scores transpose eviction (a on partitions there).
  - scoresT [t, a] computed first (N=256 rhs), then PE-transposed to [a, t]
    for softmax + the scores_raw output.
  - Weight-shared matmuls (Wr, Wq, W1) pair two batches in the rhs free dim
    (N=512) to amortize LDWEIGHTS and stay in fp32r 1-cycle/row regime.
"""

import os
import numpy as np

import concourse.bass as bass
import concourse.mybir as mybir
import concourse.tile as tile
from concourse.masks import make_identity

F32 = mybir.dt.float32
F32R = mybir.dt.float32r
I32 = mybir.dt.int32
AF = mybir.ActivationFunctionType
ALU = mybir.AluOpType
AX = mybir.AxisListType

N_CORES = 8
B, A, T, D, DQ, H = 64, 256, 128, 512, 768, 512
B_LOCAL = B // N_CORES


def _split_multi_waits(nc):
    """The walrus build here encodes at most one sync-wait per instruction.
    Tile emits instructions with several; hoist the extras into standalone
    event-semaphore waits on the same engine queue right before them."""
    for fn in nc.m.functions:
        for bb in fn.blocks:
            insts = list(bb.instructions)
            out = []
            changed = False
            for inst in insts:
                si = inst.sync_info
                if si is not None and si.on_wait and len(si.on_wait) > 1:
                    waits = list(si.on_wait)
                    for w in waits[:-1]:
                        ev = mybir.InstEventSemaphore(
                            name=f"splitwait_{nc.next_id()}",
                            engine=inst.engine,
                            ins=[], outs=[],
                            sync_info=mybir.SyncInfo(on_wait=[w], on_update=[]),
                        )
                        out.append(ev)
                    inst.sync_info = mybir.SyncInfo(
                        on_wait=[waits[-1]], on_update=list(si.on_update or []))
                    changed = True
                out.append(inst)
            if changed:
                bb.instructions = out


def build_module(b_local=B_LOCAL):
    nc = bass.Bass()

    anchor = nc.dram_tensor("anchor", [b_local, A, D], F32, kind="ExternalInput")
    anchor_t = nc.dram_tensor("anchor_t", [b_local, D, A], F32R, kind="ExternalInput")
    query = nc.dram_tensor("query", [b_local, T, DQ], F32, kind="ExternalInput")
    query_t = nc.dram_tensor("query_t", [b_local, DQ, T], F32R, kind="ExternalInput")
    mask = nc.dram_tensor("mask", [b_local, T], I32, kind="ExternalInput")
    Wr = nc.dram_tensor("Wr", [D, H], F32R, kind="ExternalInput")
    Wq = nc.dram_tensor("Wq", [DQ, H], F32R, kind="ExternalInput")
    W1 = nc.dram_tensor("W1", [D + 2 * H, H], F32R, kind="ExternalInput")
    b1 = nc.dram_tensor("b1", [H], F32, kind="ExternalInput")
    W2 = nc.dram_tensor("W2", [H, H], F32R, kind="ExternalInput")
    b2 = nc.dram_tensor("b2", [H], F32, kind="ExternalInput")
    x_out = nc.dram_tensor("x_out", [b_local, A, H], F32, kind="ExternalOutput")
    s_out = nc.dram_tensor("s_out", [b_local, A, T], F32, kind="ExternalOutput")

    KD, KQ, K1, KH = D // 128, DQ // 128, (D + 2 * H) // 128, H // 128  # 4,6,12,4

    with tile.TileContext(nc) as tc:
        from contextlib import ExitStack

        with ExitStack() as ctx:
            ec = ctx.enter_context
            consts = ec(tc.tile_pool(name="consts", bufs=1))
            pin = ec(tc.tile_pool(name="pin", bufs=2))        # pair input tiles
            pnat = ec(tc.tile_pool(name="pnat", bufs=3))      # natural-layout inputs
            pmid = ec(tc.tile_pool(name="pmid", bufs=1))      # pair intermediates
            psml = ec(tc.tile_pool(name="psml", bufs=6))      # small per-batch tiles
            pout = ec(tc.tile_pool(name="pout", bufs=4))      # output staging
            psUR = ec(tc.tile_pool(name="psUR", bufs=2, space="PSUM"))
            psX = ec(tc.tile_pool(name="psX", bufs=2, space="PSUM"))
            psB = ec(tc.tile_pool(name="psB", bufs=4, space="PSUM"))

            ident = consts.tile([128, 128], F32, tag="ident")
            make_identity(nc, ident[:])

            def load_w(t_hbm, kk, name):
                tiles = []
                for c in range(kk):
                    w = consts.tile([128, H], F32R, tag=f"{name}{c}")
                    nc.sync.dma_start(w[:], t_hbm[c * 128:(c + 1) * 128, :])
                    tiles.append(w)
                return tiles

            wr_sb = load_w(Wr, KD, "wr")
            wq_sb = load_w(Wq, KQ, "wq")
            w1_sb = w2_sb = None  # loaded after pair-0 input DMAs

            b1_sb = consts.tile([128, KH], F32, tag="b1")
            nc.sync.dma_start(b1_sb[:], b1.rearrange("(m p) -> p m", p=128))
            b2_bc = consts.tile([128, H], F32, tag="b2")
            nc.sync.dma_start(b2_bc[:], b2[None, :].to_broadcast((128, H)))

            for pr in range(b_local // 2):
                # ---- pair input tiles ----
                atr = [pin.tile([128, 2 * A], F32R, tag=f"atr{c}", name=f"atr{c}")
                       for c in range(KD)]   # anchor_t pair tiles [128, 512]
                qtp = [pin.tile([128, 2 * T], F32R, tag=f"qtp{c}", name=f"qtp{c}")
                       for c in range(KQ)]   # query_t pair tiles [128, 256]

                s_a = [[None, None], [None, None]]
                s_q = [None, None]
                negb = [None, None]

                for j in range(2):
                    g = pr * 2 + j
                    for c in range(KD):
                        nc.sync.dma_start(
                            atr[c][:, j * A:(j + 1) * A],
                            anchor_t[g, c * 128:(c + 1) * 128, :])
                    for c in range(KQ):
                        nc.sync.dma_start(
                            qtp[c][:, j * T:(j + 1) * T],
                            query_t[g, c * 128:(c + 1) * 128, :])

                    # ---- norms (natural layout) ----
                    for i in range(2):
                        a_in = pnat.tile([128, D], F32, tag="a_in")
                        nc.sync.dma_start(a_in[:], anchor[g, i * 128:(i + 1) * 128, :])
                        ss = psml.tile([128, 1], F32, tag="ss_a")
                        nc.scalar.activation(a_in[:], a_in[:], AF.Square,
                                             accum_out=ss[:])
                        srt = psml.tile([128, 1], F32, tag="srt_a")
                        nc.scalar.activation(srt[:], ss[:], AF.Sqrt)
                        sa = psml.tile([128, 1], F32, tag="sa")
                        nc.vector.reciprocal(sa[:], srt[:])
                        s_a[j][i] = sa

                    q_in = pnat.tile([128, DQ], F32, tag="q_in")
                    nc.sync.dma_start(q_in[:], query[g, :, :])
                    ssq = psml.tile([128, 1], F32, tag="ss_q")
                    nc.scalar.activation(q_in[:], q_in[:], AF.Square,
                                         accum_out=ssq[:])
                    srtq = psml.tile([128, 1], F32, tag="srt_q")
                    nc.scalar.activation(srtq[:], ssq[:], AF.Sqrt)
                    sq = psml.tile([128, 1], F32, tag="sq")
                    nc.vector.reciprocal(sq[:], srtq[:])
                    s_q[j] = sq

                    # ---- mask -> additive bias row broadcast [128, T] ----
                    mbc = psml.tile([128, T], I32, tag="mbc")
                    nc.sync.dma_start(mbc[:], mask[g][None, :].to_broadcast((128, T)))
                    nb = psml.tile([128, T], F32, tag="negb")
                    nc.gpsimd.tensor_scalar(nb[:], mbc[:], 1.0e9, -1.0e9,
                                            op0=ALU.mult, op1=ALU.add)
                    negb[j] = nb

                if w1_sb is None:
                    # deferred so pair-0 input DMAs win the queue race
                    w1_sb = load_w(W1, K1, "w1")
                    w2_sb = load_w(W2, KH, "w2")

                # ---- U_RT = (anchor_raw @ Wr)^T, pair rhs N=512 ----
                urt = []
                for m in range(KH):
                    ps = psUR.tile([128, 2 * A], F32, tag="psUR")
                    for c in range(KD):
                        nc.tensor.matmul(ps[:], (wr_sb[c][:, m * 128:(m + 1) * 128]),
                                         (atr[c][:]), start=(c == 0), stop=(c == KD - 1))
                    u = pmid.tile([128, 2 * A], F32R, tag=f"urt{m}", name=f"urt{m}", bufs=2)
                    if m % 2 == 0:
                        nc.scalar.copy(u[:], ps[:])
                    else:
                        nc.vector.tensor_copy(u[:], ps[:])
                    urt.append(u)

                # ---- UQ_T = (query_raw @ Wq)^T, pair rhs N=256 ----
                uqt = []
                for m in range(KH):
                    ps = psB.tile([128, 2 * T], F32, tag="psB")
                    for c in range(KQ):
                        nc.tensor.matmul(ps[:], (wq_sb[c][:, m * 128:(m + 1) * 128]),
                                         (qtp[c][:]), start=(c == 0), stop=(c == KQ - 1))
                    u = pmid.tile([128, 2 * T], F32R, tag=f"uqt{m}", name=f"uqt{m}", bufs=2)
                    if m % 2 == 0:
                        nc.scalar.copy(u[:], ps[:])
                    else:
                        nc.vector.tensor_copy(u[:], ps[:])
                    uqt.append(u)

                # pair tiles filled per batch below
                attnT = pmid.tile([128, 2 * A], F32R, tag="attnT")
                attsb = [pmid.tile([128, 2 * A], F32R, tag=f"attsb{m}", name=f"attsb{m}")
                         for m in range(KH)]
                prod = [pmid.tile([128, 2 * A], F32R, tag=f"prod{m}", name=f"prod{m}")
                        for m in range(KD)]

                for j in range(2):
                    g = pr * 2 + j

                    # ---- Q_proj [t, h], normalized on eviction ----
                    ps_qp = psUR.tile([128, H], F32, tag="psUR")
                    for c in range(KQ):
                        nc.tensor.matmul(ps_qp[:], (qtp[c][:, j * T:(j + 1) * T]),
                                         (wq_sb[c][:]), start=(c == 0), stop=(c == KQ - 1))
                    qp = pnat.tile([128, H], F32R, tag="qp")
                    nc.scalar.activation(qp[:], ps_qp[:], AF.Identity, bias=0.0,
                                         scale=s_q[j][:])

                    # ---- scoresT [t, a], s_q folded on eviction ----
                    ps_st = psB.tile([128, A], F32, tag="psB")
                    for m in range(KH):
                        nc.tensor.matmul(ps_st[:], (uqt[m][:, j * T:(j + 1) * T]),
                                         (urt[m][:, j * A:(j + 1) * A]),
                                         start=(m == 0), stop=(m == KH - 1))
                    st_sb = psml.tile([128, A], F32, tag="st_sb")
                    nc.scalar.activation(st_sb[:], ps_st[:], AF.Identity, bias=0.0,
                                         scale=s_q[j][:])

                    # ---- transpose to [a, t]; s_a fold; mask; softmax ----
                    for i in range(2):
                        ps_t = psB.tile([128, 128], F32, tag="psB")
                        nc.tensor.transpose(ps_t[:], st_sb[:, i * 128:(i + 1) * 128], ident[:])
                        sc_t = psml.tile([128, T], F32, tag="sc_t")
                        nc.scalar.activation(sc_t[:], ps_t[:], AF.Copy, bias=0.0,
                                             scale=s_a[j][i][:])
                        sc = pout.tile([128, T], F32, tag="sc")
                        nc.vector.tensor_add(sc[:], sc_t[:], negb[j][:])
                        nc.sync.dma_start(s_out[g, i * 128:(i + 1) * 128, :], sc[:])

                        nmax = psml.tile([128, 1], F32, tag="nmax")
                        nc.vector.reduce_max(nmax[:], sc[:], axis=AX.X, negate=True)
                        pex = psml.tile([128, T], F32, tag="pex")
                        z = psml.tile([128, 1], F32, tag="z")
                        nc.scalar.activation(pex[:], sc[:], AF.Exp, bias=nmax[:],
                                             scale=1.0, accum_out=z[:])
                        rz = psml.tile([128, 1], F32, tag="rz")
                        nc.vector.reciprocal(rz[:], z[:])
                        at = psml.tile([128, T], F32, tag="at")
                        nc.vector.tensor_scalar_mul(at[:], pex[:], rz[:])

                        ps_t2 = psB.tile([128, 128], F32, tag="psB")
                        nc.tensor.transpose(ps_t2[:], at[:], ident[:])
                        nc.scalar.copy(attnT[:, j * A + i * 128: j * A + (i + 1) * 128],
                                       ps_t2[:])

                    # ---- attendedT [h, a] ----
                    for m in range(KH):
                        ps_at = psB.tile([128, A], F32, tag="psB")
                        nc.tensor.matmul(ps_at[:], (qp[:, m * 128:(m + 1) * 128]),
                                         (attnT[:, j * A:(j + 1) * A]),
                                         start=True, stop=True)
                        if m % 2 == 0:
                            nc.scalar.copy(attsb[m][:, j * A:(j + 1) * A], ps_at[:])
                        else:
                            nc.vector.tensor_copy(attsb[m][:, j * A:(j + 1) * A], ps_at[:])
                        nc.vector.tensor_mul(prod[m][:, j * A:(j + 1) * A],
                                             atr[m][:, j * A:(j + 1) * A], ps_at[:])

                # ---- xT = relu(W1^T @ combinedT + b1), pair rhs N=512 ----
                xt = []
                for m in range(KH):
                    ps_x = psX.tile([128, 2 * A], F32, tag="psX")
                    for c in range(K1):
                        rhs = atr[c] if c < KD else (attsb[c - KD] if c < KD + KH
                                                     else prod[c - KD - KH])
                        nc.tensor.matmul(ps_x[:], (w1_sb[c][:, m * 128:(m + 1) * 128]),
                                         (rhs[:]), start=(c == 0), stop=(c == K1 - 1))
                    xm = pmid.tile([128, 2 * A], F32R, tag=f"xt{m}", name=f"xt{m}")
                    if m % 2 == 0:
                        nc.scalar.activation(xm[:], ps_x[:], AF.Relu,
                                             bias=b1_sb[:, m:m + 1], scale=1.0)
                    else:
                        nc.vector.tensor_scalar(xm[:], ps_x[:], b1_sb[:, m:m + 1], 0.0,
                                                op0=ALU.add, op1=ALU.max)
                    xt.append(xm)

                # ---- x2 = xT^T @ W2 + b2 ----
                for j in range(2):
                    g = pr * 2 + j
                    for i in range(2):
                        ps_o = psX.tile([128, H], F32, tag="psX")
                        for m in range(KH):
                            nc.tensor.matmul(
                                ps_o[:],
                                (xt[m][:, j * A + i * 128: j * A + (i + 1) * 128]),
                                (w2_sb[m][:]), start=(m == 0), stop=(m == KH - 1))
                        ob = pout.tile([128, H], F32, tag="ob")
                        nc.vector.tensor_add(ob[:], ps_o[:], b2_bc[:])
                        nc.sync.dma_start(x_out[g, i * 128:(i + 1) * 128, :], ob[:])

    return nc


_NC_CACHE = {}


def _get_nc():
    if "nc" not in _NC_CACHE:
        nc = build_module()
        _split_multi_waits(nc)  # HW-only: CoreSim chokes on raw event-sems
        _NC_CACHE["nc"] = nc
    return _NC_CACHE["nc"]


LAST_RESULTS = None


def kernel(anchor_feats, query_embs, query_mask, Wr, Wq, W1, b1, W2, b2):
    global LAST_RESULTS
    from concourse.bass_utils import run_bass_kernel_spmd

    anchor_feats = np.ascontiguousarray(np.asarray(anchor_feats, dtype=np.float32))
    query_embs = np.ascontiguousarray(np.asarray(query_embs, dtype=np.float32))
    query_mask = np.ascontiguousarray(np.asarray(query_mask, dtype=np.int32))
    anchor_t = np.ascontiguousarray(anchor_feats.transpose(0, 2, 1))
    query_t = np.ascontiguousarray(query_embs.transpose(0, 2, 1))
    weights = {
        "Wr": np.ascontiguousarray(np.asarray(Wr, dtype=np.float32)),
        "Wq": np.ascontiguousarray(np.asarray(Wq, dtype=np.float32)),
        "W1": np.ascontiguousarray(np.asarray(W1, dtype=np.float32)),
        "b1": np.ascontiguousarray(np.asarray(b1, dtype=np.float32)),
        "W2": np.ascontiguousarray(np.asarray(W2, dtype=np.float32)),
        "b2": np.ascontiguousarray(np.asarray(b2, dtype=np.float32)),
    }

    in_maps = []
    for k in range(N_CORES):
        sl = slice(k * B_LOCAL, (k + 1) * B_LOCAL)
        in_maps.append({
            "anchor": anchor_feats[sl],
            "anchor_t": anchor_t[sl],
            "query": query_embs[sl],
            "query_t": query_t[sl],
            "mask": query_mask[sl],
            **weights,
        })

    nc = _get_nc()
    extra = {}
    if os.environ.get("BASS_TMPDIR"):
        extra["tmpdir"] = os.environ["BASS_TMPDIR"]
    res = run_bass_kernel_spmd(nc, in_maps, core_ids=list(range(N_CORES)), **extra)
    LAST_RESULTS = res
    x = np.concatenate([r["x_out"] for r in res.results], axis=0)
    s = np.concatenate([r["s_out"] for r in res.results], axis=0)
    return (x, s)


# revision 24
# speedup vs baseline: 1.0511x; 1.0058x over previous
"""Trainium2 Bass kernel for CrossModalAttentionScorer.

Contract: kernel(**inputs) takes FULL inputs (B=64), returns (x, scores_raw)
matching reference.py. Data-parallel over batch across 8 NeuronCores, dense
weights replicated.

Per-core layout strategy (B_local = 8 batches, processed in 4 pairs):
  - Host pre-transposes anchor ([A,D]->[D,A]) and query ([T,Dq]->[Dq,T]) so
    every matmul contraction lands on the SBUF partition dim; no on-device
    input transposes.
  - l2-normalize folded into per-partition scales:
      s_q applied at Q_proj / scoresT PSUM evictions (t on partitions there),
      s_a applied at the scores transpose eviction (a on partitions there).
  - scoresT [t, a] computed first (N=256 rhs), then PE-transposed to [a, t]
    for softmax + the scores_raw output.
  - Weight-shared matmuls (Wr, Wq, W1) pair two batches in the rhs free dim
    (N=512) to amortize LDWEIGHTS and stay in fp32r 1-cycle/row regime.
  - DMAs are batched into single 3D-AP transfers (the SP sequencer costs
    ~0.6us per dma_start on this stack); output DMAs ride the idle GpSimd
    queue.
"""

import os
import numpy as np

import concourse.bass as bass
import concourse.mybir as mybir
import concourse.tile as tile
from concourse.masks import make_identity

F32 = mybir.dt.float32
F32R = mybir.dt.float32r
I32 = mybir.dt.int32
AF = mybir.ActivationFunctionType
ALU = mybir.AluOpType
AX = mybir.AxisListType

N_CORES = 8
B, A, T, D, DQ, H = 64, 256, 128, 512, 768, 512
B_LOCAL = B // N_CORES


def _split_multi_waits(nc):
    """The walrus build here encodes at most one sync-wait per instruction.
    Tile emits instructions with several; hoist the extras into standalone
    event-semaphore waits on the same engine queue right before them."""
    for fn in nc.m.functions:
        for bb in fn.blocks:
            insts = list(bb.instructions)
            out = []
            changed = False
            for inst in insts:
                si = inst.sync_info
                if si is not None and si.on_wait and len(si.on_wait) > 1:
                    waits = list(si.on_wait)
                    for w in waits[:-1]:
                        ev = mybir.InstEventSemaphore(
                            name=f"splitwait_{nc.next_id()}",
                            engine=inst.engine,
                            ins=[], outs=[],
                            sync_info=mybir.SyncInfo(on_wait=[w], on_update=[]),
                        )
                        out.append(ev)
                    inst.sync_info = mybir.SyncInfo(
                        on_wait=[waits[-1]], on_update=list(si.on_update or []))
                    changed = True
                out.append(inst)
            if changed:
                bb.instructions = out


def build_module(b_local=B_LOCAL):
    nc = bass.Bass()

    anchor = nc.dram_tensor("anchor", [b_local, A, D], F32, kind="ExternalInput")
    anchor_t = nc.dram_tensor("anchor_t", [b_local, D, A], F32R, kind="ExternalInput")
    query = nc.dram_tensor("query", [b_local, T, DQ], F32, kind="ExternalInput")
    query_t = nc.dram_tensor("query_t", [b_local, DQ, T], F32R, kind="ExternalInput")
    mask = nc.dram_tensor("mask", [b_local, T], I32, kind="ExternalInput")
    Wr = nc.dram_tensor("Wr", [D, H], F32R, kind="ExternalInput")
    Wq = nc.dram_tensor("Wq", [DQ, H], F32R, kind="ExternalInput")
    W1 = nc.dram_tensor("W1", [D + 2 * H, H], F32R, kind="ExternalInput")
    b1 = nc.dram_tensor("b1", [H], F32, kind="ExternalInput")
    W2 = nc.dram_tensor("W2", [H, H], F32R, kind="ExternalInput")
    b2 = nc.dram_tensor("b2", [H], F32, kind="ExternalInput")
    x_out = nc.dram_tensor("x_out", [b_local, A, H], F32, kind="ExternalOutput")
    s_out = nc.dram_tensor("s_out", [b_local, A, T], F32, kind="ExternalOutput")

    KD, KQ, K1, KH = D // 128, DQ // 128, (D + 2 * H) // 128, H // 128  # 4,6,12,4

    with tile.TileContext(nc) as tc:
        from contextlib import ExitStack

        with ExitStack() as ctx:
            ec = ctx.enter_context
            consts = ec(tc.tile_pool(name="consts", bufs=1))
            pin = ec(tc.tile_pool(name="pin", bufs=2))        # pair input tiles
            pnat = ec(tc.tile_pool(name="pnat", bufs=2))      # natural-layout inputs
            pmid = ec(tc.tile_pool(name="pmid", bufs=1))      # pair intermediates
            psml = ec(tc.tile_pool(name="psml", bufs=4))      # small per-batch tiles
            pout = ec(tc.tile_pool(name="pout", bufs=3))      # output staging
            psUR = ec(tc.tile_pool(name="psUR", bufs=2, space="PSUM"))
            psX = ec(tc.tile_pool(name="psX", bufs=2, space="PSUM"))
            psB = ec(tc.tile_pool(name="psB", bufs=4, space="PSUM"))

            ident = consts.tile([128, 128], F32, tag="ident")
            make_identity(nc, ident[:])

            def load_w(t_hbm, kk, name):
                w = consts.tile([128, kk, H], F32R, tag=name, name=name)
                nc.sync.dma_start(w[:], t_hbm.rearrange("(c p) h -> p c h", p=128))
                return w

            wr_sb = load_w(Wr, KD, "wr")
            wq_sb = load_w(Wq, KQ, "wq")
            w1_sb = w2_sb = None  # loaded after pair-0 input DMAs

            b1_sb = consts.tile([128, KH], F32, tag="b1")
            nc.sync.dma_start(b1_sb[:], b1.rearrange("(m p) -> p m", p=128))
            b2_bc = consts.tile([128, H], F32, tag="b2")
            nc.sync.dma_start(b2_bc[:], b2[None, :].to_broadcast((128, H)))

            for pr in range(b_local // 2):
                # ---- pair input tiles, one batched DMA per tensor per batch ----
                atrB = pin.tile([128, KD, 2 * A], F32R, tag="atrB")
                qtpB = pin.tile([128, KQ, 2 * T], F32R, tag="qtpB")

                s_a = [[None, None], [None, None]]
                s_q = [None, None]

                for j in range(2):
                    g = pr * 2 + j
                    nc.sync.dma_start(
                        atrB[:, :, j * A:(j + 1) * A],
                        anchor_t[g].rearrange("(c p) a -> p c a", p=128))
                    nc.sync.dma_start(
                        qtpB[:, :, j * T:(j + 1) * T],
                        query_t[g].rearrange("(c p) t -> p c t", p=128))

                    # ---- norms (natural layout) ----
                    a_in = pnat.tile([128, 2, D], F32, tag="a_in")
                    nc.sync.dma_start(
                        a_in[:], anchor[g].rearrange("(i p) d -> p i d", p=128))
                    for i in range(2):
                        ss = psml.tile([128, 1], F32, tag="ss_a")
                        nc.scalar.activation(a_in[:, i, :], a_in[:, i, :], AF.Square,
                                             accum_out=ss[:])
                        srt = psml.tile([128, 1], F32, tag="srt_a")
                        nc.scalar.activation(srt[:], ss[:], AF.Sqrt)
                        sa = psml.tile([128, 1], F32, tag="sa")
                        nc.vector.reciprocal(sa[:], srt[:])
                        s_a[j][i] = sa

                    q_in = pnat.tile([128, DQ], F32, tag="q_in")
                    nc.sync.dma_start(q_in[:], query[g, :, :])
                    ssq = psml.tile([128, 1], F32, tag="ss_q")
                    nc.scalar.activation(q_in[:], q_in[:], AF.Square,
                                         accum_out=ssq[:])
                    srtq = psml.tile([128, 1], F32, tag="srt_q")
                    nc.scalar.activation(srtq[:], ssq[:], AF.Sqrt)
                    sq = psml.tile([128, 1], F32, tag="sq")
                    nc.vector.reciprocal(sq[:], srtq[:])
                    s_q[j] = sq

                # ---- mask -> additive bias row broadcast, one DMA per pair ----
                mbc = psml.tile([128, 2, T], I32, tag="mbc")
                nc.gpsimd.dma_start(
                    mbc[:],
                    mask[pr * 2:pr * 2 + 2][None, :, :].to_broadcast((128, 2, T)))
                negb = psml.tile([128, 2, T], F32, tag="negb")
                nc.gpsimd.tensor_scalar(negb[:], mbc[:], 1.0e9, -1.0e9,
                                        op0=ALU.mult, op1=ALU.add)

                if w1_sb is None:
                    # deferred so pair-0 input DMAs win the queue race
                    w1_sb = load_w(W1, K1, "w1")
                    w2_sb = load_w(W2, KH, "w2")

                # ---- U_RT = (anchor_raw @ Wr)^T, pair rhs N=512 ----
                urt = []
                for m in range(KH):
                    ps = psUR.tile([128, 2 * A], F32, tag="psUR")
                    for c in range(KD):
                        nc.tensor.matmul(ps[:], wr_sb[:, c, m * 128:(m + 1) * 128],
                                         atrB[:, c, :], start=(c == 0),
                                         stop=(c == KD - 1))
                    u = pmid.tile([128, 2 * A], F32R, tag=f"urt{m}", name=f"urt{m}",
                                  bufs=2)
                    if m % 2 == 0:
                        nc.scalar.copy(u[:], ps[:])
                    else:
                        nc.vector.tensor_copy(u[:], ps[:])
                    urt.append(u)

                # ---- UQ_T = (query_raw @ Wq)^T, pair rhs N=256 ----
                uqt = []
                for m in range(KH):
                    ps = psB.tile([128, 2 * T], F32, tag="psB")
                    for c in range(KQ):
                        nc.tensor.matmul(ps[:], wq_sb[:, c, m * 128:(m + 1) * 128],
                                         qtpB[:, c, :], start=(c == 0),
                                         stop=(c == KQ - 1))
                    u = pmid.tile([128, 2 * T], F32R, tag=f"uqt{m}", name=f"uqt{m}",
                                  bufs=2)
                    if m % 2 == 0:
                        nc.scalar.copy(u[:], ps[:])
                    else:
                        nc.vector.tensor_copy(u[:], ps[:])
                    uqt.append(u)

                # pair tiles filled per batch below
                attnT = pmid.tile([128, 2 * A], F32R, tag="attnT")
                attsb = [pmid.tile([128, 2 * A], F32R, tag=f"attsb{m}",
                                   name=f"attsb{m}") for m in range(KH)]
                prod = [pmid.tile([128, 2 * A], F32R, tag=f"prod{m}",
                                  name=f"prod{m}") for m in range(KD)]
                scB = [None, None]

                for j in range(2):
                    g = pr * 2 + j

                    # ---- Q_proj [t, h], normalized on eviction ----
                    ps_qp = psUR.tile([128, H], F32, tag="psUR")
                    for c in range(KQ):
                        nc.tensor.matmul(ps_qp[:], qtpB[:, c, j * T:(j + 1) * T],
                                         wq_sb[:, c, :], start=(c == 0),
                                         stop=(c == KQ - 1))
                    qp = pnat.tile([128, H], F32R, tag="qp", bufs=3)
                    nc.scalar.activation(qp[:], ps_qp[:], AF.Identity, bias=0.0,
                                         scale=s_q[j][:])

                    # ---- scoresT [t, a], s_q folded on eviction ----
                    ps_st = psB.tile([128, A], F32, tag="psB")
                    for m in range(KH):
                        nc.tensor.matmul(ps_st[:], uqt[m][:, j * T:(j + 1) * T],
                                         urt[m][:, j * A:(j + 1) * A],
                                         start=(m == 0), stop=(m == KH - 1))
                    st_sb = psml.tile([128, A], F32, tag="st_sb")
                    nc.scalar.activation(st_sb[:], ps_st[:], AF.Identity, bias=0.0,
                                         scale=s_q[j][:])

                    # ---- transpose to [a, t]; s_a fold; mask; softmax ----
                    sc = pout.tile([128, 2, T], F32, tag="sc")
                    scB[j] = sc
                    for i in range(2):
                        ps_t = psB.tile([128, 128], F32, tag="psB")
                        nc.tensor.transpose(ps_t[:], st_sb[:, i * 128:(i + 1) * 128],
                                            ident[:])
                        sc_t = psml.tile([128, T], F32, tag="sc_t")
                        nc.scalar.activation(sc_t[:], ps_t[:], AF.Copy, bias=0.0,
                                             scale=s_a[j][i][:])
                        nc.vector.tensor_add(sc[:, i, :], sc_t[:], negb[:, j, :])

                        nmax = psml.tile([128, 1], F32, tag="nmax")
                        nc.vector.reduce_max(nmax[:], sc[:, i, :], axis=AX.X,
                                             negate=True)
                        pex = psml.tile([128, T], F32, tag="pex")
                        z = psml.tile([128, 1], F32, tag="z")
                        nc.scalar.activation(pex[:], sc[:, i, :], AF.Exp,
                                             bias=nmax[:], scale=1.0, accum_out=z[:])
                        rz = psml.tile([128, 1], F32, tag="rz")
                        nc.vector.reciprocal(rz[:], z[:])
                        at = psml.tile([128, T], F32, tag="at")
                        nc.vector.tensor_scalar_mul(at[:], pex[:], rz[:])

                        ps_t2 = psB.tile([128, 128], F32, tag="psB")
                        nc.tensor.transpose(ps_t2[:], at[:], ident[:])
                        nc.scalar.copy(attnT[:, j * A + i * 128: j * A + (i + 1) * 128],
                                       ps_t2[:])
                    nc.gpsimd.dma_start(
                        s_out[g].rearrange("(i p) t -> p i t", p=128), sc[:])

                    # ---- attendedT [h, a] ----
                    for m in range(KH):
                        ps_at = psB.tile([128, A], F32, tag="psB")
                        nc.tensor.matmul(ps_at[:], qp[:, m * 128:(m + 1) * 128],
                                         attnT[:, j * A:(j + 1) * A],
                                         start=True, stop=True)
                        if m % 2 == 0:
                            nc.scalar.copy(attsb[m][:, j * A:(j + 1) * A], ps_at[:])
                        else:
                            nc.vector.tensor_copy(attsb[m][:, j * A:(j + 1) * A],
                                                  ps_at[:])
                        nc.vector.tensor_mul(prod[m][:, j * A:(j + 1) * A],
                                             atrB[:, m, j * A:(j + 1) * A], ps_at[:])

                # ---- xT = relu(W1^T @ combinedT + b1), pair rhs N=512 ----
                xt = []
                for m in range(KH):
                    ps_x = psX.tile([128, 2 * A], F32, tag="psX")
                    for c in range(K1):
                        if c < KD:
                            rhs = atrB[:, c, :]
                        elif c < KD + KH:
                            rhs = attsb[c - KD][:]
                        else:
                            rhs = prod[c - KD - KH][:]
                        nc.tensor.matmul(ps_x[:], w1_sb[:, c, m * 128:(m + 1) * 128],
                                         rhs, start=(c == 0), stop=(c == K1 - 1))
                    xm = pmid.tile([128, 2 * A], F32R, tag=f"xt{m}", name=f"xt{m}")
                    if m % 2 == 0:
                        nc.scalar.activation(xm[:], ps_x[:], AF.Relu,
                                             bias=b1_sb[:, m:m + 1], scale=1.0)
                    else:
                        nc.vector.tensor_scalar(xm[:], ps_x[:], b1_sb[:, m:m + 1], 0.0,
                                                op0=ALU.add, op1=ALU.max)
                    xt.append(xm)

                # ---- x2 = xT^T @ W2 + b2 ----
                for j in range(2):
                    g = pr * 2 + j
                    ob = pout.tile([128, 2, H], F32, tag="ob")
                    for i in range(2):
                        ps_o = psX.tile([128, H], F32, tag="psX")
                        for m in range(KH):
                            nc.tensor.matmul(
                                ps_o[:],
                                xt[m][:, j * A + i * 128: j * A + (i + 1) * 128],
                                w2_sb[:, m, :], start=(m == 0), stop=(m == KH - 1))
                        if i == 0:
                            nc.vector.tensor_add(ob[:, i, :], ps_o[:], b2_bc[:])
                        else:
                            nc.vector.tensor_add(ob[:, i, :], ps_o[:], b2_bc[:])
                    nc.gpsimd.dma_start(
                        x_out[g].rearrange("(i p) h -> p i h", p=128), ob[:])

    return nc


_NC_CACHE = {}


def _get_nc():
    if "nc" not in _NC_CACHE:
        nc = build_module()
        _split_multi_waits(nc)  # HW-only: CoreSim chokes on raw event-sems
        _NC_CACHE["nc"] = nc
    return _NC_CACHE["nc"]


LAST_RESULTS = None


def kernel(anchor_feats, query_embs, query_mask, Wr, Wq, W1, b1, W2, b2):
    global LAST_RESULTS
    from concourse.bass_utils import run_bass_kernel_spmd

    anchor_feats = np.ascontiguousarray(np.asarray(anchor_feats, dtype=np.float32))
    query_embs = np.ascontiguousarray(np.asarray(query_embs, dtype=np.float32))
    query_mask = np.ascontiguousarray(np.asarray(query_mask, dtype=np.int32))
    anchor_t = np.ascontiguousarray(anchor_feats.transpose(0, 2, 1))
    query_t = np.ascontiguousarray(query_embs.transpose(0, 2, 1))
    weights = {
        "Wr": np.ascontiguousarray(np.asarray(Wr, dtype=np.float32)),
        "Wq": np.ascontiguousarray(np.asarray(Wq, dtype=np.float32)),
        "W1": np.ascontiguousarray(np.asarray(W1, dtype=np.float32)),
        "b1": np.ascontiguousarray(np.asarray(b1, dtype=np.float32)),
        "W2": np.ascontiguousarray(np.asarray(W2, dtype=np.float32)),
        "b2": np.ascontiguousarray(np.asarray(b2, dtype=np.float32)),
    }

    in_maps = []
    for k in range(N_CORES):
        sl = slice(k * B_LOCAL, (k + 1) * B_LOCAL)
        in_maps.append({
            "anchor": anchor_feats[sl],
            "anchor_t": anchor_t[sl],
            "query": query_embs[sl],
            "query_t": query_t[sl],
            "mask": query_mask[sl],
            **weights,
        })

    nc = _get_nc()
    extra = {}
    if os.environ.get("BASS_TMPDIR"):
        extra["tmpdir"] = os.environ["BASS_TMPDIR"]
    res = run_bass_kernel_spmd(nc, in_maps, core_ids=list(range(N_CORES)), **extra)
    LAST_RESULTS = res
    x = np.concatenate([r["x_out"] for r in res.results], axis=0)
    s = np.concatenate([r["s_out"] for r in res.results], axis=0)
    return (x, s)


# revision 25
# speedup vs baseline: 1.0767x; 1.0243x over previous
"""Trainium2 Bass kernel for CrossModalAttentionScorer.

Contract: kernel(**inputs) takes FULL inputs (B=64), returns (x, scores_raw)
matching reference.py. Data-parallel over batch across 8 NeuronCores, dense
weights replicated.

Per-core strategy (B_local = 8 batches, processed in 4 pairs):
  - Host pre-arranges every input partition-major ([128, ...] with each
    partition's bytes contiguous) so each DMA is 128 large-run descriptors —
    the SP sequencer on this stack pays ~5ns/descriptor of issue time, which
    made naive layouts DMA-issue-bound. One DMA per pair per tensor.
  - All matmul contractions land on the partition dim (inputs pre-transposed
    on host); no on-device input transposes.
  - l2-normalize folded into per-partition eviction scales:
      s_q at Q_proj / scoresT PSUM evictions (t on partitions there),
      s_a at the scores transpose eviction (a on partitions there).
  - scoresT [t, a] first (N=256), then PE-transpose to [a, t] for softmax
    and the scores_raw output.
  - Weight-shared matmuls (Wr, Wq, W1) put both batches of a pair in the rhs
    free dim (N=512) to amortize LDWEIGHTS; fp32r keeps 1 cycle/row.
  - Outputs staged partition-major and unscrambled on host.
"""

import os
import numpy as np

import concourse.bass as bass
import concourse.mybir as mybir
import concourse.tile as tile
from concourse.masks import make_identity

F32 = mybir.dt.float32
F32R = mybir.dt.float32r
I32 = mybir.dt.int32
AF = mybir.ActivationFunctionType
ALU = mybir.AluOpType
AX = mybir.AxisListType

N_CORES = 8
B, A, T, D, DQ, H = 64, 256, 128, 512, 768, 512
B_LOCAL = B // N_CORES
NP = B_LOCAL // 2  # pairs per core
KD, KQ, K1, KH = D // 128, DQ // 128, (D + 2 * H) // 128, H // 128  # 4,6,12,4


def _split_multi_waits(nc):
    """The walrus build here encodes at most one sync-wait per instruction.
    Tile emits instructions with several; hoist the extras into standalone
    event-semaphore waits on the same engine queue right before them."""
    for fn in nc.m.functions:
        for bb in fn.blocks:
            insts = list(bb.instructions)
            out = []
            changed = False
            for inst in insts:
                si = inst.sync_info
                if si is not None and si.on_wait and len(si.on_wait) > 1:
                    waits = list(si.on_wait)
                    for w in waits[:-1]:
                        ev = mybir.InstEventSemaphore(
                            name=f"splitwait_{nc.next_id()}",
                            engine=inst.engine,
                            ins=[], outs=[],
                            sync_info=mybir.SyncInfo(on_wait=[w], on_update=[]),
                        )
                        out.append(ev)
                    inst.sync_info = mybir.SyncInfo(
                        on_wait=[waits[-1]], on_update=list(si.on_update or []))
                    changed = True
                out.append(inst)
            if changed:
                bb.instructions = out


def build_module(n_pairs=NP):
    nc = bass.Bass()

    # All inputs partition-major; pair index outermost.
    anchor_rt = nc.dram_tensor("anchor_rt", [n_pairs, 128, 2, KD, A], F32R,
                               kind="ExternalInput")
    query_rt = nc.dram_tensor("query_rt", [n_pairs, 128, 2, KQ, T], F32R,
                              kind="ExternalInput")
    anchor_n = nc.dram_tensor("anchor_n", [n_pairs, 128, 2, 2, D], F32,
                              kind="ExternalInput")
    query_n = nc.dram_tensor("query_n", [n_pairs, 128, 2, DQ], F32,
                             kind="ExternalInput")
    mask = nc.dram_tensor("mask", [2 * n_pairs, T], I32, kind="ExternalInput")
    wr_h = nc.dram_tensor("wr_r", [128, KD, H], F32R, kind="ExternalInput")
    wq_h = nc.dram_tensor("wq_r", [128, KQ, H], F32R, kind="ExternalInput")
    w1_h = nc.dram_tensor("w1_r", [128, K1, H], F32R, kind="ExternalInput")
    w2_h = nc.dram_tensor("w2_r", [128, KH, H], F32R, kind="ExternalInput")
    b1_h = nc.dram_tensor("b1_r", [128, KH], F32, kind="ExternalInput")
    b2_h = nc.dram_tensor("b2", [H], F32, kind="ExternalInput")
    x_out = nc.dram_tensor("x_outP", [2 * n_pairs, 128, 2, H], F32,
                           kind="ExternalOutput")
    s_out = nc.dram_tensor("s_outP", [2 * n_pairs, 128, 2, T], F32,
                           kind="ExternalOutput")

    with tile.TileContext(nc) as tc:
        from contextlib import ExitStack

        with ExitStack() as ctx:
            ec = ctx.enter_context
            consts = ec(tc.tile_pool(name="consts", bufs=1))
            pin = ec(tc.tile_pool(name="pin", bufs=2))        # pair input tiles
            pnat = ec(tc.tile_pool(name="pnat", bufs=2))      # natural-layout inputs
            pmid = ec(tc.tile_pool(name="pmid", bufs=1))      # pair intermediates
            psml = ec(tc.tile_pool(name="psml", bufs=4))      # small per-batch tiles
            pout = ec(tc.tile_pool(name="pout", bufs=3))      # output staging
            psUR = ec(tc.tile_pool(name="psUR", bufs=2, space="PSUM"))
            psX = ec(tc.tile_pool(name="psX", bufs=2, space="PSUM"))
            psB = ec(tc.tile_pool(name="psB", bufs=4, space="PSUM"))

            ident = consts.tile([128, 128], F32, tag="ident")
            make_identity(nc, ident[:])

            wr_sb = consts.tile([128, KD, H], F32R, tag="wr")
            nc.sync.dma_start(wr_sb[:], wr_h[:])
            wq_sb = consts.tile([128, KQ, H], F32R, tag="wq")
            nc.sync.dma_start(wq_sb[:], wq_h[:])
            w1_sb = w2_sb = None  # deferred until after pair-0 input DMAs

            b1_sb = consts.tile([128, KH], F32, tag="b1")
            nc.sync.dma_start(b1_sb[:], b1_h[:])
            b2_bc = consts.tile([128, H], F32, tag="b2")
            nc.sync.dma_start(b2_bc[:], b2_h[None, :].to_broadcast((128, H)))

            for pr in range(n_pairs):
                # ---- pair inputs: one DMA per tensor per pair ----
                atrB = pin.tile([128, 2, KD, A], F32R, tag="atrB")
                nc.sync.dma_start(atrB[:], anchor_rt[pr])
                qtpB = pin.tile([128, 2, KQ, T], F32R, tag="qtpB")
                nc.sync.dma_start(qtpB[:], query_rt[pr])
                a_in = pnat.tile([128, 2, 2, D], F32, tag="a_in")
                nc.sync.dma_start(a_in[:], anchor_n[pr])
                q_in = pnat.tile([128, 2, DQ], F32, tag="q_in")
                nc.sync.dma_start(q_in[:], query_n[pr])

                if w1_sb is None:
                    w1_sb = consts.tile([128, K1, H], F32R, tag="w1")
                    nc.sync.dma_start(w1_sb[:], w1_h[:])
                    w2_sb = consts.tile([128, KH, H], F32R, tag="w2")
                    nc.sync.dma_start(w2_sb[:], w2_h[:])

                # ---- norms ----
                s_a = [[None, None], [None, None]]
                s_q = [None, None]
                for j in range(2):
                    for i in range(2):
                        ss = psml.tile([128, 1], F32, tag="ss_a")
                        nc.scalar.activation(a_in[:, j, i, :], a_in[:, j, i, :],
                                             AF.Square, accum_out=ss[:])
                        srt = psml.tile([128, 1], F32, tag="srt_a")
                        nc.scalar.activation(srt[:], ss[:], AF.Sqrt)
                        sa = psml.tile([128, 1], F32, tag="sa")
                        nc.vector.reciprocal(sa[:], srt[:])
                        s_a[j][i] = sa
                    ssq = psml.tile([128, 1], F32, tag="ss_q")
                    nc.scalar.activation(q_in[:, j, :], q_in[:, j, :], AF.Square,
                                         accum_out=ssq[:])
                    srtq = psml.tile([128, 1], F32, tag="srt_q")
                    nc.scalar.activation(srtq[:], ssq[:], AF.Sqrt)
                    sq = psml.tile([128, 1], F32, tag="sq")
                    nc.vector.reciprocal(sq[:], srtq[:])
                    s_q[j] = sq

                # ---- mask -> additive bias row broadcast, one DMA per pair ----
                mbc = psml.tile([128, 2, T], I32, tag="mbc")
                nc.gpsimd.dma_start(
                    mbc[:],
                    mask[pr * 2:pr * 2 + 2][None, :, :].to_broadcast((128, 2, T)))
                negb = psml.tile([128, 2, T], F32, tag="negb")
                nc.gpsimd.tensor_scalar(negb[:], mbc[:], 1.0e9, -1.0e9,
                                        op0=ALU.mult, op1=ALU.add)

                # ---- U_RT = (anchor_raw @ Wr)^T, pair rhs N=512 ----
                urt = []
                for m in range(KH):
                    ps = psUR.tile([128, 2 * A], F32, tag="psUR")
                    for c in range(KD):
                        nc.tensor.matmul(ps[:], wr_sb[:, c, m * 128:(m + 1) * 128],
                                         atrB[:, :, c, :], start=(c == 0),
                                         stop=(c == KD - 1))
                    u = pmid.tile([128, 2 * A], F32R, tag=f"urt{m}", name=f"urt{m}",
                                  bufs=2)
                    if m % 2 == 0:
                        nc.scalar.copy(u[:], ps[:])
                    else:
                        nc.vector.tensor_copy(u[:], ps[:])
                    urt.append(u)

                # ---- UQ_T = (query_raw @ Wq)^T, pair rhs N=256 ----
                uqt = []
                for m in range(KH):
                    ps = psB.tile([128, 2 * T], F32, tag="psB")
                    for c in range(KQ):
                        nc.tensor.matmul(ps[:], wq_sb[:, c, m * 128:(m + 1) * 128],
                                         qtpB[:, :, c, :], start=(c == 0),
                                         stop=(c == KQ - 1))
                    u = pmid.tile([128, 2 * T], F32R, tag=f"uqt{m}", name=f"uqt{m}",
                                  bufs=2)
                    if m % 2 == 0:
                        nc.scalar.copy(u[:], ps[:])
                    else:
                        nc.vector.tensor_copy(u[:], ps[:])
                    uqt.append(u)

                # pair tiles filled per batch below
                attnT = pmid.tile([128, 2 * A], F32R, tag="attnT")
                attsb = [pmid.tile([128, 2 * A], F32R, tag=f"attsb{m}",
                                   name=f"attsb{m}") for m in range(KH)]
                prod = [pmid.tile([128, 2 * A], F32R, tag=f"prod{m}",
                                  name=f"prod{m}") for m in range(KD)]

                for j in range(2):
                    g = pr * 2 + j

                    # ---- Q_proj [t, h], normalized on eviction ----
                    ps_qp = psUR.tile([128, H], F32, tag="psUR")
                    for c in range(KQ):
                        nc.tensor.matmul(ps_qp[:], qtpB[:, j, c, :],
                                         wq_sb[:, c, :], start=(c == 0),
                                         stop=(c == KQ - 1))
                    qp = pnat.tile([128, H], F32R, tag="qp", bufs=3)
                    nc.scalar.activation(qp[:], ps_qp[:], AF.Identity, bias=0.0,
                                         scale=s_q[j][:])

                    # ---- scoresT [t, a], s_q folded on eviction ----
                    ps_st = psB.tile([128, A], F32, tag="psB")
                    for m in range(KH):
                        nc.tensor.matmul(ps_st[:], uqt[m][:, j * T:(j + 1) * T],
                                         urt[m][:, j * A:(j + 1) * A],
                                         start=(m == 0), stop=(m == KH - 1))
                    st_sb = psml.tile([128, A], F32, tag="st_sb")
                    nc.scalar.activation(st_sb[:], ps_st[:], AF.Identity, bias=0.0,
                                         scale=s_q[j][:])

                    # ---- transpose to [a, t]; s_a fold; mask; softmax ----
                    sc = pout.tile([128, 2, T], F32, tag="sc")
                    for i in range(2):
                        ps_t = psB.tile([128, 128], F32, tag="psB")
                        nc.tensor.transpose(ps_t[:], st_sb[:, i * 128:(i + 1) * 128],
                                            ident[:])
                        sc_t = psml.tile([128, T], F32, tag="sc_t")
                        nc.scalar.activation(sc_t[:], ps_t[:], AF.Copy, bias=0.0,
                                             scale=s_a[j][i][:])
                        nc.vector.tensor_add(sc[:, i, :], sc_t[:], negb[:, j, :])

                        nmax = psml.tile([128, 1], F32, tag="nmax")
                        nc.vector.reduce_max(nmax[:], sc[:, i, :], axis=AX.X,
                                             negate=True)
                        pex = psml.tile([128, T], F32, tag="pex")
                        z = psml.tile([128, 1], F32, tag="z")
                        nc.scalar.activation(pex[:], sc[:, i, :], AF.Exp,
                                             bias=nmax[:], scale=1.0, accum_out=z[:])
                        rz = psml.tile([128, 1], F32, tag="rz")
                        nc.vector.reciprocal(rz[:], z[:])
                        at = psml.tile([128, T], F32, tag="at")
                        nc.vector.tensor_scalar_mul(at[:], pex[:], rz[:])

                        ps_t2 = psB.tile([128, 128], F32, tag="psB")
                        nc.tensor.transpose(ps_t2[:], at[:], ident[:])
                        nc.scalar.copy(attnT[:, j * A + i * 128: j * A + (i + 1) * 128],
                                       ps_t2[:])
                    nc.gpsimd.dma_start(s_out[g], sc[:])

                    # ---- attendedT [h, a] ----
                    for m in range(KH):
                        ps_at = psB.tile([128, A], F32, tag="psB")
                        nc.tensor.matmul(ps_at[:], qp[:, m * 128:(m + 1) * 128],
                                         attnT[:, j * A:(j + 1) * A],
                                         start=True, stop=True)
                        if m % 2 == 0:
                            nc.scalar.copy(attsb[m][:, j * A:(j + 1) * A], ps_at[:])
                        else:
                            nc.vector.tensor_copy(attsb[m][:, j * A:(j + 1) * A],
                                                  ps_at[:])
                        nc.vector.tensor_mul(prod[m][:, j * A:(j + 1) * A],
                                             atrB[:, j, m, :], ps_at[:])

                # ---- xT = relu(W1^T @ combinedT + b1), pair rhs N=512 ----
                xt = []
                for m in range(KH):
                    ps_x = psX.tile([128, 2 * A], F32, tag="psX")
                    for c in range(K1):
                        if c < KD:
                            rhs = atrB[:, :, c, :]
                        elif c < KD + KH:
                            rhs = attsb[c - KD][:]
                        else:
                            rhs = prod[c - KD - KH][:]
                        nc.tensor.matmul(ps_x[:], w1_sb[:, c, m * 128:(m + 1) * 128],
                                         rhs, start=(c == 0), stop=(c == K1 - 1))
                    xm = pmid.tile([128, 2 * A], F32R, tag=f"xt{m}", name=f"xt{m}")
                    if m % 2 == 0:
                        nc.scalar.activation(xm[:], ps_x[:], AF.Relu,
                                             bias=b1_sb[:, m:m + 1], scale=1.0)
                    else:
                        nc.vector.tensor_scalar(xm[:], ps_x[:], b1_sb[:, m:m + 1], 0.0,
                                                op0=ALU.add, op1=ALU.max)
                    xt.append(xm)

                # ---- x2 = xT^T @ W2 + b2 ----
                for j in range(2):
                    g = pr * 2 + j
                    ob = pout.tile([128, 2, H], F32, tag="ob")
                    for i in range(2):
                        ps_o = psX.tile([128, H], F32, tag="psX")
                        for m in range(KH):
                            nc.tensor.matmul(
                                ps_o[:],
                                xt[m][:, j * A + i * 128: j * A + (i + 1) * 128],
                                w2_sb[:, m, :], start=(m == 0), stop=(m == KH - 1))
                        nc.vector.tensor_add(ob[:, i, :], ps_o[:], b2_bc[:])
                    nc.gpsimd.dma_start(x_out[g], ob[:])

    return nc


_NC_CACHE = {}


def _get_nc():
    if "nc" not in _NC_CACHE:
        nc = build_module()
        _split_multi_waits(nc)  # HW-only: CoreSim chokes on raw event-sems
        _NC_CACHE["nc"] = nc
    return _NC_CACHE["nc"]


LAST_RESULTS = None


def _prep(anchor_feats, query_embs):
    """Host-side partition-major packing (pure layout, no math)."""
    nb = anchor_feats.shape[0]
    npairs = nb // 2
    at = anchor_feats.transpose(0, 2, 1)                   # [B, D, A]
    anchor_rt = np.ascontiguousarray(
        at.reshape(npairs, 2, KD, 128, A).transpose(0, 3, 1, 2, 4))
    qt = query_embs.transpose(0, 2, 1)                     # [B, DQ, T]
    query_rt = np.ascontiguousarray(
        qt.reshape(npairs, 2, KQ, 128, T).transpose(0, 3, 1, 2, 4))
    anchor_n = np.ascontiguousarray(
        anchor_feats.reshape(npairs, 2, 2, 128, D).transpose(0, 3, 1, 2, 4))
    query_n = np.ascontiguousarray(
        query_embs.reshape(npairs, 2, 128, DQ).transpose(0, 2, 1, 3))
    return anchor_rt, query_rt, anchor_n, query_n


def kernel(anchor_feats, query_embs, query_mask, Wr, Wq, W1, b1, W2, b2):
    global LAST_RESULTS
    from concourse.bass_utils import run_bass_kernel_spmd

    anchor_feats = np.ascontiguousarray(np.asarray(anchor_feats, dtype=np.float32))
    query_embs = np.ascontiguousarray(np.asarray(query_embs, dtype=np.float32))
    query_mask = np.ascontiguousarray(np.asarray(query_mask, dtype=np.int32))
    anchor_rt, query_rt, anchor_n, query_n = _prep(anchor_feats, query_embs)

    def wprep(w, kk):
        w = np.asarray(w, dtype=np.float32)
        return np.ascontiguousarray(w.reshape(kk, 128, H).transpose(1, 0, 2))

    weights = {
        "wr_r": wprep(Wr, KD),
        "wq_r": wprep(Wq, KQ),
        "w1_r": wprep(W1, K1),
        "w2_r": wprep(W2, KH),
        "b1_r": np.ascontiguousarray(
            np.asarray(b1, dtype=np.float32).reshape(KH, 128).T),
        "b2": np.ascontiguousarray(np.asarray(b2, dtype=np.float32)),
    }

    in_maps = []
    for k in range(N_CORES):
        bs = slice(k * B_LOCAL, (k + 1) * B_LOCAL)
        ps = slice(k * NP, (k + 1) * NP)
        in_maps.append({
            "anchor_rt": anchor_rt[ps],
            "query_rt": query_rt[ps],
            "anchor_n": anchor_n[ps],
            "query_n": query_n[ps],
            "mask": query_mask[bs],
            **weights,
        })

    nc = _get_nc()
    extra = {}
    if os.environ.get("BASS_TMPDIR"):
        extra["tmpdir"] = os.environ["BASS_TMPDIR"]
    res = run_bass_kernel_spmd(nc, in_maps, core_ids=list(range(N_CORES)), **extra)
    LAST_RESULTS = res
    x = np.concatenate(
        [r["x_outP"].transpose(0, 2, 1, 3).reshape(B_LOCAL, A, H)
         for r in res.results], axis=0)
    s = np.concatenate(
        [r["s_outP"].transpose(0, 2, 1, 3).reshape(B_LOCAL, A, T)
         for r in res.results], axis=0)
    return (x, s)
